# revision 4
# baseline (speedup 1.0000x reference)
"""Causal self-attention on 8 Trainium2 NeuronCores (Bass/Tile).

Problem: x[4, 2048, 1024], w_in[3072, 1024], w_out[1024, 1024], 16 heads.
    qkv = x @ w_in.T ; per-(b,h) causal softmax attention ; out = y @ w_out.T

Sharding (SPMD — one program, per-core input data):
    core c  ->  batch b = c // 2, head-group g = c % 2 (heads 8g .. 8g+7).
    Each core projects q/k/v for its 8 heads of its batch and runs causal
    attention for them.  The pair (2b, 2b+1) AllGathers the two head-group
    halves of yT (per head-pair; the last pair per q-chunk so the output
    projection overlaps the tail of attention), then each core computes the
    output projection for half of the output features (core even: e_out
    0..511, odd: 512..1023) over all 2048 tokens of its batch.

All on-chip compute is bf16 (fp32 PSUM accumulation).  Feature-major
layouts throughout; softmax denominators come from a ones-column PREPENDED
to V (AV matmul has M = 65, denominator on PSUM partition 0) so
normalization is recip-at-partition-0 + K=1 matmul broadcast + multiply —
no DMA round trips in the PE's dependency cone.

Engine/queue plumbing (the v2 -> v3 wins):
  * DMAs are split across the two HWDGE queues by purpose: SP carries
    xT (in qc-column chunks so the first q-chunk lands early), y writes and
    the pair-3 gather chunks; ACT carries the weights, the whole-pair yg
    reloads and the outT writes.  One queue is strict FIFO, so v2's single
    queue head-of-line-blocked the output phase ~50 us.
  * Out-proj contraction order [0,1,2,4,5,6,3,7] puts the pair-3 k-tiles
    last so 6/8 of every accumulation group is ready before the final
    AllGather chunk — out-proj fills the PE during pair-3's ACT deficit.
  * The V projection is emitted in 4-tile chunks between pair-0's q-chunk
    blocks: spare PE work the scheduler uses wherever ACT (exp) is behind.
"""

import sys

for _p in ("/opt/trn_rl_repo",):
    if _p not in sys.path:
        sys.path.insert(0, _p)

import numpy as np
import ml_dtypes

BF16 = ml_dtypes.bfloat16

B, S, D = 4, 2048, 1024
H, HD = 16, 64
N_CORES = 8
HPC = 8            # heads per core
NPAIRS = HPC // 2  # head pairs per core
QC = S // 512      # q-chunks per head
TT = S // 128      # token tiles
DT = D // 128      # feature (d) tiles
EHALF = D // 2     # output features per core

_PROG = None       # cached compiled program


def _build_program():
    import concourse.bass as bass
    from concourse import bacc
    import concourse.tile as tile
    import concourse.mybir as mybir
    from concourse.bass import broadcast_tensor_aps
    from contextlib import ExitStack

    f32 = mybir.dt.float32
    bf16 = mybir.dt.bfloat16
    AF = mybir.ActivationFunctionType
    OP = mybir.AluOpType

    nc = bacc.Bacc("TRN2", target_bir_lowering=False, debug=False,
                   num_devices=N_CORES)

    xT = nc.dram_tensor("xT", [D, S], bf16, kind="ExternalInput").ap()
    wqkT = nc.dram_tensor("wqkT", [D, 2 * HPC * HD], bf16,
                          kind="ExternalInput").ap()
    wvT = nc.dram_tensor("wvT", [D, HPC * HD], bf16, kind="ExternalInput").ap()
    woT = nc.dram_tensor("woT", [D, EHALF], bf16, kind="ExternalInput").ap()
    tri = nc.dram_tensor("tri", [128, 128], bf16, kind="ExternalInput").ap()
    outT = nc.dram_tensor("outT", [EHALF, S], f32, kind="ExternalOutput").ap()

    # per-pair local y (pairs 0..2 whole, pair 3 in per-qc chunks)
    y_locp = [nc.dram_tensor(f"y_loc{i}", [128, S], bf16)
              for i in range(NPAIRS - 1)]
    y_loc3 = [nc.dram_tensor(f"y_loc3_{qc}", [128, 512], bf16)
              for qc in range(QC)]
    y_gat = [nc.dram_tensor(f"y_gat{i}", [2, 128, S], bf16)
             for i in range(NPAIRS - 1)]
    y_gat3 = [nc.dram_tensor(f"y_gat3_{qc}", [2, 128, 512], bf16)
              for qc in range(QC)]

    with tile.TileContext(nc) as tc:
        def mm(out, lhsT, rhs, start, stop):
            nc.tensor.matmul(out, lhsT, rhs, start=start, stop=stop)

        with ExitStack() as perm:
            const_pool = perm.enter_context(tc.tile_pool(name="const", bufs=1))
            v_pool = perm.enter_context(tc.tile_pool(name="vsb", bufs=TT))
            wv_pool = perm.enter_context(tc.tile_pool(name="wvsb", bufs=DT))
            mm_ps = perm.enter_context(
                tc.tile_pool(name="mmps", bufs=2, space="PSUM"))
            qk_pool = perm.enter_context(tc.tile_pool(name="qksb", bufs=6))
            xt_pool = perm.enter_context(tc.tile_pool(name="xtsb", bufs=DT))
            wqk_pool = perm.enter_context(
                tc.tile_pool(name="wqksb", bufs=3 * DT))
            p_pool = perm.enter_context(tc.tile_pool(name="psb", bufs=3))
            n_pool = perm.enter_context(tc.tile_pool(name="nsb", bufs=2))
            sc_ps = perm.enter_context(
                tc.tile_pool(name="scps", bufs=2, space="PSUM"))
            y_ps = perm.enter_context(
                tc.tile_pool(name="yps", bufs=2, space="PSUM"))
            wo_pool = perm.enter_context(tc.tile_pool(name="wosb", bufs=DT))
            yg_pool = perm.enter_context(tc.tile_pool(name="ygsb", bufs=DT))
            o_pool = perm.enter_context(tc.tile_pool(name="osb", bufs=2))

            # ---- DMA issue: weights on the ACT HWDGE queue, xT (in
            # qc-column chunks, first q-chunk first) on the SP queue ----
            tri_sb = const_pool.tile([128, 128], bf16, tag="tri")
            nc.scalar.dma_start(tri_sb[:], tri[:])
            ones_sb = const_pool.tile([128, 65], bf16, tag="ones")
            nc.gpsimd.memset(ones_sb[:], 1.0)

            wv_sb = [wv_pool.tile([128, HPC * HD], bf16, tag="wv",
                                  name=f"wv{d}") for d in range(DT)]
            for d in range(DT):
                nc.scalar.dma_start(wv_sb[d][:], wvT[d * 128:(d + 1) * 128, :])

            wqk_sb = [[wqk_pool.tile([128, 256], bf16, tag="wqk",
                                     name=f"wqk{i}_{d}") for d in range(DT)]
                      for i in range(NPAIRS)]

            def load_wqk(i):
                for d in range(DT):
                    nc.scalar.dma_start(
                        wqk_sb[i][d][:, 0:128],
                        wqkT[d * 128:(d + 1) * 128, i * 128:(i + 1) * 128])
                    nc.scalar.dma_start(
                        wqk_sb[i][d][:, 128:256],
                        wqkT[d * 128:(d + 1) * 128,
                             (NPAIRS + i) * 128:(NPAIRS + i + 1) * 128])

            load_wqk(0)

            xt_sb = [xt_pool.tile([128, S], bf16, tag="xt", name=f"xt{d}")
                     for d in range(DT)]
            for cc in range(QC):
                for d in range(DT):
                    nc.sync.dma_start(
                        xt_sb[d][:, cc * 512:(cc + 1) * 512],
                        xT[d * 128:(d + 1) * 128, cc * 512:(cc + 1) * 512])

            load_wqk(1)
            wo_sb = [wo_pool.tile([128, EHALF], bf16, tag="wo",
                                  name=f"wo{d}") for d in range(DT)]
            for d in range(DT):
                nc.scalar.dma_start(wo_sb[d][:], woT[d * 128:(d + 1) * 128, :])

            # v_sb[t]: [128, 8*65] — per head a ones column THEN 64 v cols
            v_sb = [v_pool.tile([128, HPC * (HD + 1)], bf16, tag="v",
                                name=f"v{t}") for t in range(TT)]

            def vproj(trange):
                for t in trange:
                    ps = mm_ps.tile([128, 512], f32, tag="mm")
                    for d in range(DT):
                        mm(ps[:], xt_sb[d][:, t * 128:(t + 1) * 128],
                           wv_sb[d][:], start=(d == 0), stop=(d == DT - 1))
                    vdst = v_sb[t][:].rearrange(
                        "p (h e) -> p h e", h=HPC)[:, :, 1:HD + 1]
                    vsrc = ps[:].rearrange("p (h e) -> p h e", h=HPC)
                    nc.vector.tensor_copy(vdst, vsrc)
                    nc.vector.tensor_copy(
                        v_sb[t][:].rearrange(
                            "p (h e) -> p h e", h=HPC)[:, :, 0:1],
                        ones_sb[:, 0:HPC].unsqueeze(-1))

            qk_sb = {}

            def qkproj(i):
                q_sb = qk_pool.tile([128, S], bf16, tag="qk", name=f"q{i}")
                k_sb = qk_pool.tile([128, S], bf16, tag="qk", name=f"k{i}")
                qk_sb[i] = (q_sb, k_sb)
                for which, dest in ((0, q_sb), (1, k_sb)):
                    for qc in range(QC):
                        ps = mm_ps.tile([128, 512], f32, tag="mm")
                        for d in range(DT):
                            mm(ps[:],
                               wqk_sb[i][d][:, which * 128:(which + 1) * 128],
                               xt_sb[d][:, qc * 512:(qc + 1) * 512],
                               start=(d == 0), stop=(d == DT - 1))
                        nc.vector.tensor_copy(
                            dest[:, qc * 512:(qc + 1) * 512], ps[:])

            def attn_qc(i, qc):
                q_sb, k_sb = qk_sb[i]
                nkt = 4 * qc + 4   # causal: k-tiles 0 .. 4qc+3
                yps = [y_ps.tile([HD + 1, 512], f32, tag="yt",
                                 name=f"yps{i}_{qc}_{h}")
                       for h in range(2)]
                for kt in range(nkt):
                    j = kt - 4 * qc
                    lo = max(0, j) * 128
                    sc = sc_ps.tile([128, 1024], f32, tag="sc")
                    pt = p_pool.tile([128, 1024], bf16, tag="p")
                    for h in range(2):
                        mm(sc[:, h * 512 + lo:(h + 1) * 512],
                           k_sb[h * 64:(h + 1) * 64,
                                kt * 128:(kt + 1) * 128],
                           q_sb[h * 64:(h + 1) * 64,
                                qc * 512 + lo:(qc + 1) * 512],
                           start=True, stop=True)
                    # exp(score / 8) for both heads in one ACT call
                    src = sc[:].rearrange("p (s c) -> p s c", s=2)[
                        :, :, lo:512]
                    dst = pt[:].rearrange("p (s c) -> p s c", s=2)[
                        :, :, lo:512]
                    nc.scalar.activation(dst, src, AF.Exp, scale=0.125)
                    if j >= 0:   # mask the diagonal band (both heads)
                        band = pt[:].rearrange("p (s c) -> p s c", s=2)[
                            :, :, lo:lo + 128]
                        trib = tri_sb[:].unsqueeze(1)
                        band_b, tri_b = broadcast_tensor_aps(band, trib)
                        nc.vector.tensor_mul(band_b, band_b, tri_b)
                    for h in range(2):
                        hl = 2 * i + h
                        mm(yps[h][:, lo:512],
                           v_sb[kt][:, hl * 65:hl * 65 + 65],
                           pt[:, h * 512 + lo:(h + 1) * 512],
                           start=(kt == 0), stop=(kt == nkt - 1))
                # normalize: y[1:65] * (1 / y[0]) and stream to DRAM
                for h in range(2):
                    ysc = n_pool.tile([65, 512], f32, tag="ysc")
                    nc.vector.tensor_copy(ysc[:], yps[h][:])
                    rcp = n_pool.tile([1, 512], f32, tag="rcp")
                    nc.vector.reciprocal_approx_fast(
                        out=rcp[:], in_=ysc[0:1, :])
                    rcpr = n_pool.tile([1, 512], bf16, tag="rcpr")
                    nc.vector.tensor_copy(rcpr[:], rcp[:])
                    rb = mm_ps.tile([65, 512], f32, tag="mm")
                    mm(rb[:], ones_sb[0:1, 0:65], rcpr[:],
                       start=True, stop=True)
                    nout = n_pool.tile([65, 512], bf16, tag="nout")
                    nc.vector.tensor_mul(nout[:], ysc[:], rb[:])
                    if i < NPAIRS - 1:
                        nc.sync.dma_start(
                            y_locp[i][h * 64:(h + 1) * 64,
                                      qc * 512:(qc + 1) * 512],
                            nout[1:65, :])
                    else:
                        nc.sync.dma_start(
                            y_loc3[qc][h * 64:(h + 1) * 64, :],
                            nout[1:65, :])

            # yg k-tile k: half = k // NPAIRS, pair = k % NPAIRS
            yg_sb = [yg_pool.tile([128, S], bf16, tag="yg", name=f"yg{k}")
                     for k in range(DT)]

            def gather(i):
                if i < NPAIRS - 1:
                    nc.gpsimd.collective_compute(
                        "AllGather", OP.bypass,
                        replica_groups=[[0, 1], [2, 3], [4, 5], [6, 7]],
                        ins=[y_locp[i][:]],
                        outs=[y_gat[i][:]])
                    # whole-pair yg reloads on the ACT queue (idle mid-run)
                    for half in range(2):
                        nc.scalar.dma_start(yg_sb[half * NPAIRS + i][:],
                                            y_gat[i][half])
                else:
                    for qc in range(QC):
                        nc.gpsimd.collective_compute(
                            "AllGather", OP.bypass,
                            replica_groups=[[0, 1], [2, 3], [4, 5], [6, 7]],
                            ins=[y_loc3[qc][:]],
                            outs=[y_gat3[qc][:]])
                        # pair-3 chunks ride the SP queue right behind the
                        # nout writes that feed them
                        for half in range(2):
                            nc.sync.dma_start(
                                yg_sb[half * NPAIRS + i][
                                    :, qc * 512:(qc + 1) * 512],
                                y_gat3[qc][half])

            # ---------------- schedule ----------------
            qkproj(0)
            vproj(range(0, 4))
            attn_qc(0, 0)
            vproj(range(4, 8))
            attn_qc(0, 1)
            vproj(range(8, 12))
            attn_qc(0, 2)
            vproj(range(12, 16))
            attn_qc(0, 3)
            gather(0)
            for i in range(1, NPAIRS):
                qkproj(i)
                if i + 1 < NPAIRS:
                    load_wqk(i + 1)
                for qc in range(QC):
                    attn_qc(i, qc)
                gather(i)

            # ---------------- output projection ----------------
            # pair-3 k-tiles (3, 7) accumulate last so 6/8 of each group is
            # spare PE work before the final gather chunk lands
            korder = [0, 1, 2, 4, 5, 6, 3, 7]
            for tch in range(QC):
                for m in range(EHALF // 128):
                    ps = mm_ps.tile([128, 512], f32, tag="mm")
                    for ki, k in enumerate(korder):
                        mm(ps[:], wo_sb[k][:, m * 128:(m + 1) * 128],
                           yg_sb[k][:, tch * 512:(tch + 1) * 512],
                           start=(ki == 0), stop=(ki == DT - 1))
                    ob = o_pool.tile([128, 512], f32, tag="o")
                    nc.vector.tensor_copy(ob[:], ps[:])
                    nc.scalar.dma_start(
                        outT[m * 128:(m + 1) * 128,
                             tch * 512:(tch + 1) * 512], ob[:])
    nc.finalize()
    return nc


def _prep_inputs(x, w_in, w_out):
    """Build per-core input maps (host-side sharding, bf16)."""
    x = np.asarray(x, dtype=np.float32)
    w_in = np.asarray(w_in, dtype=np.float32)
    w_out = np.asarray(w_out, dtype=np.float32)

    tri = np.triu(np.ones((128, 128), dtype=np.float32))  # 1 where k <= q
    tri16 = tri.astype(BF16)
    in_maps = []
    for c in range(N_CORES):
        b, g = c // 2, c % 2
        heads = [8 * g + h for h in range(HPC)]
        xTb = np.ascontiguousarray(x[b].T.astype(BF16))          # [D, S]
        # wqkT: cols i*128 -> q rows of heads (8g+2i, 8g+2i+1); then k pairs
        qcols, kcols = [], []
        for i in range(NPAIRS):
            hA, hB = heads[2 * i], heads[2 * i + 1]
            qcols.append(w_in[hA * HD:(hA + 1) * HD, :])
            qcols.append(w_in[hB * HD:(hB + 1) * HD, :])
            kcols.append(w_in[D + hA * HD:D + (hA + 1) * HD, :])
            kcols.append(w_in[D + hB * HD:D + (hB + 1) * HD, :])
        wqkT = np.ascontiguousarray(
            np.concatenate(qcols + kcols, axis=0).T.astype(BF16))  # [D, 1024]
        wvT = np.ascontiguousarray(np.concatenate(
            [w_in[2 * D + h * HD:2 * D + (h + 1) * HD, :] for h in heads],
            axis=0).T.astype(BF16))                                # [D, 512]
        woT = np.ascontiguousarray(
            w_out[g * EHALF:(g + 1) * EHALF, :].T.astype(BF16))    # [D, 512]
        in_maps.append({
            "xT": xTb, "wqkT": wqkT, "wvT": wvT, "woT": woT, "tri": tri16,
        })
    return in_maps


def kernel(x, w_in, w_out):
    global _PROG
    from concourse.bass_utils import run_bass_kernel_spmd

    if _PROG is None:
        _PROG = _build_program()
    in_maps = _prep_inputs(x, w_in, w_out)
    res = run_bass_kernel_spmd(_PROG, in_maps, list(range(N_CORES)))

    out = np.empty((B, S, D), dtype=np.float32)
    for c in range(N_CORES):
        b, g = c // 2, c % 2
        out[b, :, g * EHALF:(g + 1) * EHALF] = res.results[c]["outT"].T
    return out


# revision 5
# speedup vs baseline: 1.0308x; 1.0308x over previous
"""Causal self-attention on 8 Trainium2 NeuronCores (Bass/Tile).

Problem: x[4, 2048, 1024], w_in[3072, 1024], w_out[1024, 1024], 16 heads.
    qkv = x @ w_in.T ; per-(b,h) causal softmax attention ; out = y @ w_out.T

Sharding (SPMD — one program, per-core input data):
    core c  ->  batch b = c // 2, head-group g = c % 2 (heads 8g .. 8g+7).
    Each core projects q/k/v for its 8 heads of its batch and runs causal
    attention for them.  The pair (2b, 2b+1) AllGathers the two head-group
    halves of yT (per head-pair; the last pair per q-chunk), then each core
    computes the output projection for half of the output features (core
    even: e_out 0..511, odd: 512..1023) over all 2048 tokens of its batch.

All on-chip compute is bf16 (fp32 PSUM accumulation).  Feature-major
layouts throughout; softmax denominators come from a ones-column PREPENDED
to V (AV matmul has M = 65, denominator on PSUM partition 0) so
normalization is recip-at-partition-0 + K=1 matmul broadcast + multiply.

Scheduling: the per-engine programs are static and in-order, so all "fill"
work is EMITTED interleaved at q-chunk granularity:
  * pair i's attention q-chunk blocks are followed by pair i+1's q/k
    projection chunks (and, for pair 0, the V-projection tile chunks) —
    the PE consumes them exactly where the exp (ACT) pipeline is behind,
    and the DVE cast that materializes q/k lands mid-pair instead of
    serializing at the pair boundary;
  * the output projection is split per (tch, m) into group A over y
    k-tiles {0,1,4,5} (head-pairs 0/1, gathered early — runs as fill
    during pairs 2-3) accumulated to SBUF, and group B over {2,6,3,7}
    finished with a DVE add at the tail, so no PSUM tile ever idles
    waiting for the final AllGather chunk.
DMA queues: SP carries xT (qc-column chunks, first q-chunk first), late
wqk, y writes, yg reloads; ACT carries startup weights and outT writes.
"""

import sys

for _p in ("/opt/trn_rl_repo",):
    if _p not in sys.path:
        sys.path.insert(0, _p)

import numpy as np
import ml_dtypes

BF16 = ml_dtypes.bfloat16

B, S, D = 4, 2048, 1024
H, HD = 16, 64
N_CORES = 8
HPC = 8            # heads per core
NPAIRS = HPC // 2  # head pairs per core
QC = S // 512      # q-chunks per head
TT = S // 128      # token tiles
DT = D // 128      # feature (d) tiles
EHALF = D // 2     # output features per core

_PROG = None       # cached compiled program


def _build_program():
    import concourse.bass as bass
    from concourse import bacc
    import concourse.tile as tile
    import concourse.mybir as mybir
    from concourse.bass import broadcast_tensor_aps
    from contextlib import ExitStack

    f32 = mybir.dt.float32
    bf16 = mybir.dt.bfloat16
    AF = mybir.ActivationFunctionType
    OP = mybir.AluOpType

    nc = bacc.Bacc("TRN2", target_bir_lowering=False, debug=False,
                   num_devices=N_CORES)

    xT = nc.dram_tensor("xT", [D, S], bf16, kind="ExternalInput").ap()
    # wqkT packed per pair: cols [i*256, i*256+256) = q(128) | k(128)
    wqkT = nc.dram_tensor("wqkT", [D, 2 * HPC * HD], bf16,
                          kind="ExternalInput").ap()
    wvT = nc.dram_tensor("wvT", [D, HPC * HD], bf16, kind="ExternalInput").ap()
    woT = nc.dram_tensor("woT", [D, EHALF], bf16, kind="ExternalInput").ap()
    tri = nc.dram_tensor("tri", [128, 128], bf16, kind="ExternalInput").ap()
    outT = nc.dram_tensor("outT", [EHALF, S], f32, kind="ExternalOutput").ap()

    # per-pair local y (pairs 0..2 whole, pair 3 in per-qc chunks)
    y_locp = [nc.dram_tensor(f"y_loc{i}", [128, S], bf16)
              for i in range(NPAIRS - 1)]
    y_loc3 = [nc.dram_tensor(f"y_loc3_{qc}", [128, 512], bf16)
              for qc in range(QC)]
    y_gat = [nc.dram_tensor(f"y_gat{i}", [2, 128, S], bf16)
             for i in range(NPAIRS - 1)]
    y_gat3 = [nc.dram_tensor(f"y_gat3_{qc}", [2, 128, 512], bf16)
              for qc in range(QC)]

    with tile.TileContext(nc) as tc:
        def mm(out, lhsT, rhs, start, stop):
            nc.tensor.matmul(out, lhsT, rhs, start=start, stop=stop)

        with ExitStack() as perm:
            const_pool = perm.enter_context(tc.tile_pool(name="const", bufs=1))
            v_pool = perm.enter_context(tc.tile_pool(name="vsb", bufs=TT))
            wv_pool = perm.enter_context(tc.tile_pool(name="wvsb", bufs=DT))
            mm_ps = perm.enter_context(
                tc.tile_pool(name="mmps", bufs=2, space="PSUM"))
            qk_pool = perm.enter_context(tc.tile_pool(name="qksb", bufs=6))
            xt_pool = perm.enter_context(tc.tile_pool(name="xtsb", bufs=DT))
            wqk_pool = perm.enter_context(
                tc.tile_pool(name="wqksb", bufs=3 * DT))
            p_pool = perm.enter_context(tc.tile_pool(name="psb", bufs=3))
            n_pool = perm.enter_context(tc.tile_pool(name="nsb", bufs=2))
            sc_ps = perm.enter_context(
                tc.tile_pool(name="scps", bufs=2, space="PSUM"))
            y_ps = perm.enter_context(
                tc.tile_pool(name="yps", bufs=2, space="PSUM"))
            wo_pool = perm.enter_context(tc.tile_pool(name="wosb", bufs=DT))
            yg_pool = perm.enter_context(tc.tile_pool(name="ygsb", bufs=DT))
            pa_pool = perm.enter_context(
                tc.tile_pool(name="pasb", bufs=QC * EHALF // 128))
            o_pool = perm.enter_context(tc.tile_pool(name="osb", bufs=3))

            # ---- DMA issue.  ACT HWDGE queue: tri, wqk0, wv, wqk1, wo
            # (startup, ACT idle).  SP queue: xT in qc chunks. ----
            tri_sb = const_pool.tile([128, 128], bf16, tag="tri")
            nc.scalar.dma_start(tri_sb[:], tri[:])
            ones_sb = const_pool.tile([128, 65], bf16, tag="ones")
            nc.gpsimd.memset(ones_sb[:], 1.0)

            wqk_sb = [[wqk_pool.tile([128, 256], bf16, tag="wqk",
                                     name=f"wqk{i}_{d}") for d in range(DT)]
                      for i in range(NPAIRS)]

            def load_wqk(i, eng):
                for d in range(DT):
                    eng.dma_start(
                        wqk_sb[i][d][:],
                        wqkT[d * 128:(d + 1) * 128, i * 256:(i + 1) * 256])

            load_wqk(0, nc.scalar)

            wv_sb = [wv_pool.tile([128, HPC * HD], bf16, tag="wv",
                                  name=f"wv{d}") for d in range(DT)]
            for d in range(DT):
                nc.scalar.dma_start(wv_sb[d][:], wvT[d * 128:(d + 1) * 128, :])

            xt_sb = [xt_pool.tile([128, S], bf16, tag="xt", name=f"xt{d}")
                     for d in range(DT)]
            for cc in range(QC):
                for d in range(DT):
                    nc.sync.dma_start(
                        xt_sb[d][:, cc * 512:(cc + 1) * 512],
                        xT[d * 128:(d + 1) * 128, cc * 512:(cc + 1) * 512])

            load_wqk(1, nc.scalar)
            wo_sb = [wo_pool.tile([128, EHALF], bf16, tag="wo",
                                  name=f"wo{d}") for d in range(DT)]
            for d in range(DT):
                nc.scalar.dma_start(wo_sb[d][:], woT[d * 128:(d + 1) * 128, :])

            # v_sb[t]: [128, 8*65] — per head a ones column THEN 64 v cols
            v_sb = [v_pool.tile([128, HPC * (HD + 1)], bf16, tag="v",
                                name=f"v{t}") for t in range(TT)]

            def vproj(trange):
                for t in trange:
                    ps = mm_ps.tile([128, 512], f32, tag="mm")
                    for d in range(DT):
                        mm(ps[:], xt_sb[d][:, t * 128:(t + 1) * 128],
                           wv_sb[d][:], start=(d == 0), stop=(d == DT - 1))
                    vdst = v_sb[t][:].rearrange(
                        "p (h e) -> p h e", h=HPC)[:, :, 1:HD + 1]
                    vsrc = ps[:].rearrange("p (h e) -> p h e", h=HPC)
                    nc.vector.tensor_copy(vdst, vsrc)
                    nc.vector.tensor_copy(
                        v_sb[t][:].rearrange(
                            "p (h e) -> p h e", h=HPC)[:, :, 0:1],
                        ones_sb[:, 0:HPC].unsqueeze(-1))

            qk_sb = {}

            def qkproj_alloc(i):
                q_sb = qk_pool.tile([128, S], bf16, tag="qk", name=f"q{i}")
                k_sb = qk_pool.tile([128, S], bf16, tag="qk", name=f"k{i}")
                qk_sb[i] = (q_sb, k_sb)

            def qkproj_chunk(i, which, qc):
                dest = qk_sb[i][which]
                ps = mm_ps.tile([128, 512], f32, tag="mm")
                for d in range(DT):
                    mm(ps[:],
                       wqk_sb[i][d][:, which * 128:(which + 1) * 128],
                       xt_sb[d][:, qc * 512:(qc + 1) * 512],
                       start=(d == 0), stop=(d == DT - 1))
                nc.vector.tensor_copy(dest[:, qc * 512:(qc + 1) * 512], ps[:])

            def qkproj_all(i):
                qkproj_alloc(i)
                for which in (0, 1):
                    for qc in range(QC):
                        qkproj_chunk(i, which, qc)

            def attn_qc(i, qc):
                q_sb, k_sb = qk_sb[i]
                nkt = 4 * qc + 4   # causal: k-tiles 0 .. 4qc+3
                yps = [y_ps.tile([HD + 1, 512], f32, tag="yt",
                                 name=f"yps{i}_{qc}_{h}")
                       for h in range(2)]
                for kt in range(nkt):
                    j = kt - 4 * qc
                    lo = max(0, j) * 128
                    sc = sc_ps.tile([128, 1024], f32, tag="sc")
                    pt = p_pool.tile([128, 1024], bf16, tag="p")
                    for h in range(2):
                        mm(sc[:, h * 512 + lo:(h + 1) * 512],
                           k_sb[h * 64:(h + 1) * 64,
                                kt * 128:(kt + 1) * 128],
                           q_sb[h * 64:(h + 1) * 64,
                                qc * 512 + lo:(qc + 1) * 512],
                           start=True, stop=True)
                    # exp(score / 8) for both heads in one ACT call
                    src = sc[:].rearrange("p (s c) -> p s c", s=2)[
                        :, :, lo:512]
                    dst = pt[:].rearrange("p (s c) -> p s c", s=2)[
                        :, :, lo:512]
                    nc.scalar.activation(dst, src, AF.Exp, scale=0.125)
                    if j >= 0:   # mask the diagonal band (both heads)
                        band = pt[:].rearrange("p (s c) -> p s c", s=2)[
                            :, :, lo:lo + 128]
                        trib = tri_sb[:].unsqueeze(1)
                        band_b, tri_b = broadcast_tensor_aps(band, trib)
                        nc.vector.tensor_mul(band_b, band_b, tri_b)
                    for h in range(2):
                        hl = 2 * i + h
                        mm(yps[h][:, lo:512],
                           v_sb[kt][:, hl * 65:hl * 65 + 65],
                           pt[:, h * 512 + lo:(h + 1) * 512],
                           start=(kt == 0), stop=(kt == nkt - 1))
                # normalize: y[1:65] * (1 / y[0]) and stream to DRAM
                for h in range(2):
                    ysc = n_pool.tile([65, 512], f32, tag="ysc")
                    nc.vector.tensor_copy(ysc[:], yps[h][:])
                    rcp = n_pool.tile([1, 512], f32, tag="rcp")
                    nc.vector.reciprocal_approx_fast(
                        out=rcp[:], in_=ysc[0:1, :])
                    rcpr = n_pool.tile([1, 512], bf16, tag="rcpr")
                    nc.vector.tensor_copy(rcpr[:], rcp[:])
                    rb = mm_ps.tile([65, 512], f32, tag="mm")
                    mm(rb[:], ones_sb[0:1, 0:65], rcpr[:],
                       start=True, stop=True)
                    nout = n_pool.tile([65, 512], bf16, tag="nout")
                    nc.vector.tensor_mul(nout[:], ysc[:], rb[:])
                    if i < NPAIRS - 1:
                        nc.sync.dma_start(
                            y_locp[i][h * 64:(h + 1) * 64,
                                      qc * 512:(qc + 1) * 512],
                            nout[1:65, :])
                    else:
                        nc.sync.dma_start(
                            y_loc3[qc][h * 64:(h + 1) * 64, :],
                            nout[1:65, :])

            # yg k-tile k: half = k // NPAIRS, pair = k % NPAIRS
            yg_sb = [yg_pool.tile([128, S], bf16, tag="yg", name=f"yg{k}")
                     for k in range(DT)]
            RG = [[0, 1], [2, 3], [4, 5], [6, 7]]

            def gather(i):
                nc.gpsimd.collective_compute(
                    "AllGather", OP.bypass, replica_groups=RG,
                    ins=[y_locp[i][:]], outs=[y_gat[i][:]])
                for half in range(2):
                    nc.sync.dma_start(yg_sb[half * NPAIRS + i][:],
                                      y_gat[i][half])

            def gather3_qc(qc):
                nc.gpsimd.collective_compute(
                    "AllGather", OP.bypass, replica_groups=RG,
                    ins=[y_loc3[qc][:]], outs=[y_gat3[qc][:]])
                for half in range(2):
                    nc.sync.dma_start(
                        yg_sb[half * NPAIRS + NPAIRS - 1][
                            :, qc * 512:(qc + 1) * 512],
                        y_gat3[qc][half])

            # out-proj split: group A over k {0,1,4,5} -> SBUF partial
            # (pure fill, ready once pairs 0-1 are gathered); group B over
            # k {2,6,3,7} -> PSUM + DVE add at the tail
            KA, KB = [0, 1, 4, 5], [2, 6, 3, 7]
            pa_sb = {}

            def oproj_a(tch, m):
                ps = mm_ps.tile([128, 512], f32, tag="mm")
                for ki, k in enumerate(KA):
                    mm(ps[:], wo_sb[k][:, m * 128:(m + 1) * 128],
                       yg_sb[k][:, tch * 512:(tch + 1) * 512],
                       start=(ki == 0), stop=(ki == len(KA) - 1))
                pa = pa_pool.tile([128, 512], bf16, tag="pa",
                                  name=f"pa{tch}_{m}")
                pa_sb[(tch, m)] = pa
                nc.vector.tensor_copy(pa[:], ps[:])

            def oproj_b(tch, m):
                ps = mm_ps.tile([128, 512], f32, tag="mm")
                for ki, k in enumerate(KB):
                    mm(ps[:], wo_sb[k][:, m * 128:(m + 1) * 128],
                       yg_sb[k][:, tch * 512:(tch + 1) * 512],
                       start=(ki == 0), stop=(ki == len(KB) - 1))
                ob = o_pool.tile([128, 512], f32, tag="o")
                nc.vector.tensor_add(ob[:], pa_sb[(tch, m)][:], ps[:])
                nc.scalar.dma_start(
                    outT[m * 128:(m + 1) * 128,
                         tch * 512:(tch + 1) * 512], ob[:])

            # ---------------- schedule ----------------
            # pair 0: qk proj up front, V projection + pair-1 proj chunks
            # interleaved between its attention q-chunk blocks
            qkproj_all(0)
            vproj(range(0, 4))
            qkproj_alloc(1)
            attn_qc(0, 0)
            vproj(range(4, 8))
            qkproj_chunk(1, 0, 0)
            qkproj_chunk(1, 1, 0)
            attn_qc(0, 1)
            vproj(range(8, 12))
            qkproj_chunk(1, 0, 1)
            qkproj_chunk(1, 1, 1)
            attn_qc(0, 2)
            vproj(range(12, 16))
            qkproj_chunk(1, 0, 2)
            qkproj_chunk(1, 1, 2)
            attn_qc(0, 3)
            qkproj_chunk(1, 0, 3)
            qkproj_chunk(1, 1, 3)
            gather(0)
            load_wqk(2, nc.sync)

            # pair 1
            qkproj_alloc(2)
            for qc in range(QC):
                attn_qc(1, qc)
                qkproj_chunk(2, 0, qc)
                qkproj_chunk(2, 1, qc)
            gather(1)
            load_wqk(3, nc.sync)

            # pair 2 (group-A out-proj fill becomes ready mid-pair)
            qkproj_alloc(3)
            a_fill = [(tch, m) for tch in range(QC)
                      for m in range(EHALF // 128)]
            for qc in range(QC):
                attn_qc(2, qc)
                qkproj_chunk(3, 0, qc)
                qkproj_chunk(3, 1, qc)
                if qc >= 2:
                    for _ in range(3):
                        if a_fill:
                            oproj_a(*a_fill.pop(0))
            gather(2)

            # pair 3: per-qc gather + group-A fill
            for qc in range(QC):
                attn_qc(3, qc)
                gather3_qc(qc)
                nfill = 3 if qc < 2 else 2
                for _ in range(nfill):
                    if a_fill:
                        oproj_a(*a_fill.pop(0))

            while a_fill:
                oproj_a(*a_fill.pop(0))

            # tail: group B + add + store
            for tch in range(QC):
                for m in range(EHALF // 128):
                    oproj_b(tch, m)
    nc.finalize()
    return nc


def _prep_inputs(x, w_in, w_out):
    """Build per-core input maps (host-side sharding, bf16)."""
    x = np.asarray(x, dtype=np.float32)
    w_in = np.asarray(w_in, dtype=np.float32)
    w_out = np.asarray(w_out, dtype=np.float32)

    tri = np.triu(np.ones((128, 128), dtype=np.float32))  # 1 where k <= q
    tri16 = tri.astype(BF16)
    in_maps = []
    for c in range(N_CORES):
        b, g = c // 2, c % 2
        heads = [8 * g + h for h in range(HPC)]
        xTb = np.ascontiguousarray(x[b].T.astype(BF16))          # [D, S]
        # wqkT packed per pair i: q rows of heads (8g+2i, 8g+2i+1), then
        # the same heads' k rows -> cols [i*256, (i+1)*256) on device
        blocks = []
        for i in range(NPAIRS):
            hA, hB = heads[2 * i], heads[2 * i + 1]
            blocks.append(w_in[hA * HD:(hA + 1) * HD, :])
            blocks.append(w_in[hB * HD:(hB + 1) * HD, :])
            blocks.append(w_in[D + hA * HD:D + (hA + 1) * HD, :])
            blocks.append(w_in[D + hB * HD:D + (hB + 1) * HD, :])
        wqkT = np.ascontiguousarray(
            np.concatenate(blocks, axis=0).T.astype(BF16))         # [D, 1024]
        wvT = np.ascontiguousarray(np.concatenate(
            [w_in[2 * D + h * HD:2 * D + (h + 1) * HD, :] for h in heads],
            axis=0).T.astype(BF16))                                # [D, 512]
        woT = np.ascontiguousarray(
            w_out[g * EHALF:(g + 1) * EHALF, :].T.astype(BF16))    # [D, 512]
        in_maps.append({
            "xT": xTb, "wqkT": wqkT, "wvT": wvT, "woT": woT, "tri": tri16,
        })
    return in_maps


def kernel(x, w_in, w_out):
    global _PROG
    from concourse.bass_utils import run_bass_kernel_spmd

    if _PROG is None:
        _PROG = _build_program()
    in_maps = _prep_inputs(x, w_in, w_out)
    res = run_bass_kernel_spmd(_PROG, in_maps, list(range(N_CORES)))

    out = np.empty((B, S, D), dtype=np.float32)
    for c in range(N_CORES):
        b, g = c // 2, c % 2
        out[b, :, g * EHALF:(g + 1) * EHALF] = res.results[c]["outT"].T
    return out
